# revision 6
# baseline (speedup 1.0000x reference)
"""Trainium2 Bass kernel for the VQ-autoencoder problem (nn_DCN_9474697855729).

Reference computation (see problem statement):
  embedding      = MLP_enc(x)          # 1024 -> 2048 -> 1024 -> 512 -> 256, relu on hidden
  reconstruction = MLP_dec(embedding)  # 256 -> 512 -> 1024 -> 2048 -> 1024, relu on hidden
  labels         = one_hot(argmin_k ||embedding - centers_k||^2)

Sharding: data-parallel over batch across 8 NeuronCores (weights + centers
replicated). Each core processes R = 8192/8 = 1024 rows.

Device layout: activations are kept feature-major ("transposed", [feat, row])
in SBUF so that every layer is out[f_out, r] = W[f_in, f_out].T @ h[f_in, r]
with the DRAM weights already in lhsT orientation. The host pre-transposes
x and the weights into DMA-friendly layouts (free on CPU) and undoes the
transpose on the outputs.

For the codebook: argmin_k (e2 + c2 - 2 e.c) == argmin_k (c2 - 2 e.c), so we
matmul emb.T against (-2*centers.T) into [row, center] tiles, add the
precomputed c2 row (replicated across partitions on the host), reduce-min
along the free axis and emit labels = (score == rowmin) as int32.
"""

import numpy as np
from contextlib import ExitStack

import concourse.bass as bass
import concourse.tile as tile
from concourse import bacc, mybir
from concourse.bass_utils import run_bass_kernel_spmd

F32 = mybir.dt.float32
I32 = mybir.dt.int32

B, D_IN, D_EMB, K = 8192, 1024, 256, 1024
N_CORES = 8
R = B // N_CORES  # rows per core

# (input_key_W, input_key_b, fan_in, fan_out, relu)
LAYERS = [
    ("We0", "be0", 1024, 2048, True),
    ("We1", "be1", 2048, 1024, True),
    ("We2", "be2", 1024, 512, True),
    ("We3", "be3", 512, 256, False),
    ("Wd0", "bd0", 256, 512, True),
    ("Wd1", "bd1", 512, 1024, True),
    ("Wd2", "bd2", 1024, 2048, True),
    ("Wd3", "bd3", 2048, 1024, False),
]

# activation tile tags cycle so non-overlapping lifetimes share SBUF slots
ACT_TAGS = ["actA", "actB", "actA", "actB", "actE", "actA", "actB", "actA", "actB"]

NSLICES = [slice(0, 512), slice(512, 1024)]

_CACHED_NC = None


def _build_module(loops=1):
    nc = bacc.Bacc("TRN2", target_bir_lowering=False, debug=False, num_devices=N_CORES)

    xT = nc.dram_tensor("xT", [D_IN // 128, 128, R], F32, kind="ExternalInput").ap()
    w_aps, b_aps = [], []
    for li, (_, _, fi, fo, _) in enumerate(LAYERS):
        ks, ms = fi // 128, fo // 128
        w_aps.append(
            nc.dram_tensor(f"w{li}", [ms, 128, ks, 128], F32, kind="ExternalInput").ap()
        )
        b_aps.append(nc.dram_tensor(f"b{li}", [128, ms], F32, kind="ExternalInput").ap())
    crhs = nc.dram_tensor("crhs", [D_EMB // 128, 128, K], F32, kind="ExternalInput").ap()
    c2b = nc.dram_tensor("c2b", [128, K], F32, kind="ExternalInput").ap()

    recT_o = nc.dram_tensor("recT", [D_IN // 128, 128, R], F32, kind="ExternalOutput").ap()
    embT_o = nc.dram_tensor("embT", [D_EMB // 128, 128, R], F32, kind="ExternalOutput").ap()
    lab_o = nc.dram_tensor("labels", [R // 128, 128, K], I32, kind="ExternalOutput").ap()

    with tile.TileContext(nc) as tc, ExitStack() as ctx:
        acts = ctx.enter_context(tc.tile_pool(name="acts", bufs=1))
        wpool = ctx.enter_context(tc.tile_pool(name="wpool", bufs=3))
        bpool = ctx.enter_context(tc.tile_pool(name="bpool", bufs=2))
        psum = ctx.enter_context(tc.tile_pool(name="psum", bufs=6, space="PSUM"))
        cb = ctx.enter_context(tc.tile_pool(name="cb", bufs=1))
        dpool = ctx.enter_context(tc.tile_pool(name="dpool", bufs=2))
        lpool = ctx.enter_context(tc.tile_pool(name="lpool", bufs=2))
        mpool = ctx.enter_context(tc.tile_pool(name="mpool", bufs=2))

        def mlp_layer(li, a_in, rep):
            _, _, fi, fo, relu = LAYERS[li]
            ks, ms = fi // 128, fo // 128
            a_out = acts.tile(
                [128, ms, R], F32, tag=ACT_TAGS[li + 1], name=f"act{li}_{rep}"
            )
            bt = bpool.tile([128, ms], F32, tag="bias", name=f"b{li}_{rep}_sb")
            nc.sync.dma_start(bt[:], b_aps[li][:])
            func = (
                mybir.ActivationFunctionType.Relu
                if relu
                else mybir.ActivationFunctionType.Identity
            )
            for m in range(ms):
                wt = wpool.tile([128, ks, 128], F32, tag="w", name=f"w{li}_{m}_{rep}")
                nc.sync.dma_start(wt[:], w_aps[li][m])
                for n in range(2):
                    ps = psum.tile(
                        [128, 512], F32, tag="ps", name=f"ps{li}_{m}_{n}_{rep}"
                    )
                    for k in range(ks):
                        nc.tensor.matmul(
                            ps[:],
                            wt[:, k, :],
                            a_in[:, k, NSLICES[n]],
                            start=(k == 0),
                            stop=(k == ks - 1),
                        )
                    nc.scalar.activation(
                        a_out[:, m, NSLICES[n]], ps[:], func, bias=bt[:, m : m + 1]
                    )
            return a_out

        for rep in range(loops):
            # input activations, feature-major
            a_x = acts.tile([128, D_IN // 128, R], F32, tag=ACT_TAGS[0], name=f"x_sb_{rep}")
            for c in range(D_IN // 128):
                nc.sync.dma_start(a_x[:, c, :], xT[c])

            # codebook constants
            crhs_sb = cb.tile([128, D_EMB // 128, K], F32, tag="crhs", name=f"crhs_sb_{rep}")
            for k in range(D_EMB // 128):
                nc.sync.dma_start(crhs_sb[:, k, :], crhs[k])
            c2b_sb = cb.tile([128, K], F32, tag="c2b", name=f"c2b_sb_{rep}")
            nc.sync.dma_start(c2b_sb[:], c2b[:])

            h = a_x
            for li in range(4):
                h = mlp_layer(li, h, rep)
            emb = h

            # embedding output
            for k in range(D_EMB // 128):
                nc.sync.dma_start(embT_o[k], emb[:, k, :])

            # codebook: score[row, center] = c2[center] - 2*e.c (argmin-equivalent)
            for m in range(R // 128):
                dist = dpool.tile([128, K], F32, tag="dist", name=f"dist{m}_{rep}")
                for n in range(2):
                    ps = psum.tile([128, 512], F32, tag="ps", name=f"cps{m}_{n}_{rep}")
                    for k in range(D_EMB // 128):
                        nc.tensor.matmul(
                            ps[:],
                            emb[:, k, m * 128 : (m + 1) * 128],
                            crhs_sb[:, k, NSLICES[n]],
                            start=(k == 0),
                            stop=(k == D_EMB // 128 - 1),
                        )
                    nc.vector.tensor_add(
                        dist[:, NSLICES[n]], ps[:], c2b_sb[:, NSLICES[n]]
                    )
                mn = mpool.tile([128, 1], F32, tag="mn", name=f"mn{m}_{rep}")
                nc.vector.tensor_reduce(
                    mn[:], dist[:], axis=mybir.AxisListType.X, op=mybir.AluOpType.min
                )
                lab = lpool.tile([128, K], I32, tag="lab", name=f"lab{m}_{rep}")
                nc.vector.tensor_scalar(
                    lab[:], dist[:], mn[:], None, mybir.AluOpType.is_equal
                )
                nc.sync.dma_start(lab_o[m], lab[:])

            # decoder
            for li in range(4, 8):
                h = mlp_layer(li, h, rep)
            rec = h
            for c in range(D_IN // 128):
                nc.sync.dma_start(recT_o[c], rec[:, c, :])

    nc.compile()
    return nc


def _get_nc():
    global _CACHED_NC
    if _CACHED_NC is None:
        _CACHED_NC = _build_module()
    return _CACHED_NC


def _host_prep(inputs):
    """Build the DMA-friendly host-side arrays shared by all cores."""
    shared = {}
    for li, (wk, bk, fi, fo, _) in enumerate(LAYERS):
        ks, ms = fi // 128, fo // 128
        W = np.ascontiguousarray(np.asarray(inputs[wk], dtype=np.float32))
        # [ks,128,ms,128] -> [ms,128(part=fan_in%128),ks,128(fan_out chunk)]
        shared[f"w{li}"] = np.ascontiguousarray(
            W.reshape(ks, 128, ms, 128).transpose(2, 1, 0, 3)
        )
        b = np.asarray(inputs[bk], dtype=np.float32)
        shared[f"b{li}"] = np.ascontiguousarray(b.reshape(ms, 128).T)
    C = np.asarray(inputs["centers"], dtype=np.float32)
    c2 = (C * C).sum(axis=1).astype(np.float32)  # [K]
    shared["crhs"] = np.ascontiguousarray(
        (-2.0 * C.T).reshape(D_EMB // 128, 128, K).astype(np.float32)
    )
    shared["c2b"] = np.ascontiguousarray(np.broadcast_to(c2[None, :], (128, K)))
    return shared


def kernel(**inputs):
    x = np.asarray(inputs["x"], dtype=np.float32)
    assert x.shape == (B, D_IN)

    shared = _host_prep(inputs)
    in_maps = []
    for c in range(N_CORES):
        shard = x[c * R : (c + 1) * R]  # [R, D_IN]
        xT = np.ascontiguousarray(shard.T).reshape(D_IN // 128, 128, R)
        m = dict(shared)
        m["xT"] = xT
        in_maps.append(m)

    nc = _get_nc()
    res = run_bass_kernel_spmd(nc, in_maps, core_ids=list(range(N_CORES)))

    recs, embs, labs = [], [], []
    for c in range(N_CORES):
        out = res.results[c]
        recs.append(out["recT"].reshape(D_IN, R).T)  # [R, 1024]
        embs.append(out["embT"].reshape(D_EMB, R).T)  # [R, 256]
        labs.append(out["labels"].reshape(R, K))  # [R, 1024] int32
    reconstruction = np.ascontiguousarray(np.concatenate(recs, axis=0), dtype=np.float32)
    embedding = np.ascontiguousarray(np.concatenate(embs, axis=0), dtype=np.float32)
    labels = np.ascontiguousarray(np.concatenate(labs, axis=0), dtype=np.int32)
    return reconstruction, embedding, labels


# revision 14
# speedup vs baseline: 1.6965x; 1.6965x over previous
"""Trainium2 Bass kernel for the VQ-autoencoder problem (nn_DCN_9474697855729).

Reference computation (see problem statement):
  embedding      = MLP_enc(x)          # 1024 -> 2048 -> 1024 -> 512 -> 256, relu on hidden
  reconstruction = MLP_dec(embedding)  # 256 -> 512 -> 1024 -> 2048 -> 1024, relu on hidden
  labels         = one_hot(argmin_k ||embedding - centers_k||^2)

Sharding: data-parallel over batch across 8 NeuronCores (weights + centers
replicated). Each core processes R = 8192/8 = 1024 rows.

Device layout: activations are kept feature-major ("transposed", [feat, row])
in SBUF so that every layer is out[f_out, r] = W[f_in, f_out].T @ h[f_in, r]
with the DRAM weights already in lhsT orientation. The host pre-transposes
x and the weights into DMA-friendly layouts (free on CPU) and undoes the
transpose on the outputs.

For the codebook: argmin_k (e2 + c2 - 2 e.c) == argmin_k (c2 - 2 e.c), so we
matmul emb.T against (-2*centers.T) into [row, center] tiles, add the
precomputed c2 row (replicated across partitions on the host), reduce-min
along the free axis and emit labels = (score == rowmin) as int32.
"""

import numpy as np
from contextlib import ExitStack

import concourse.bass as bass
import concourse.tile as tile
from concourse import bacc, mybir
from concourse.bass_utils import run_bass_kernel_spmd

F32 = mybir.dt.float32
F32R = mybir.dt.float32r
I32 = mybir.dt.int32

B, D_IN, D_EMB, K = 8192, 1024, 256, 1024
N_CORES = 8
R = B // N_CORES  # rows per core

# (input_key_W, input_key_b, fan_in, fan_out, relu)
LAYERS = [
    ("We0", "be0", 1024, 2048, True),
    ("We1", "be1", 2048, 1024, True),
    ("We2", "be2", 1024, 512, True),
    ("We3", "be3", 512, 256, False),
    ("Wd0", "bd0", 256, 512, True),
    ("Wd1", "bd1", 512, 1024, True),
    ("Wd2", "bd2", 1024, 2048, True),
    ("Wd3", "bd3", 2048, 1024, False),
]

# Matmul precision per layer. The encoder (layers 0-3) must stay exact fp32:
# the codebook argmin needs the embedding accurate to ~1e-6 (min top-2
# relative gap measured at 1.3e-5). The decoder only feeds the
# reconstruction output, where float32r's ~1.3e-4 relative error is well
# within tolerance — and f32r matmuls run at 4x the fp32 rate.
# f32r semantics (measured): both operands RNE-rounded to 11 mantissa bits,
# exact multiply, fp32 PSUM accumulate. The BIR verifier requires f32r
# matmul operands to be produced as f32r, so decoder hidden activations are
# written as f32r by the ACT engine and decoder weights are host-rounded.
LAYER_F32R = [False, False, False, False, True, True, True, True]
# Whether layer li's OUTPUT activation tile is written as f32r (True when the
# next layer's matmul is f32r and the tile isn't needed at full precision).
ACT_OUT_F32R = [False, False, False, False, True, True, True, False]


def _rne11(x):
    """Round fp32 array to 11 explicit mantissa bits (f32r's input rounding)."""
    xi = np.ascontiguousarray(x, dtype=np.float32).view(np.uint32)
    shift = np.uint32(12)
    mask = np.uint32(0xFFFFFFFF) << shift
    add = np.uint32(1 << 11)
    lsb = (xi >> shift) & np.uint32(1)
    out = ((xi + add - np.uint32(1) + lsb) & mask).view(np.float32)
    return out

# activation tile tags cycle so non-overlapping lifetimes share SBUF slots
ACT_TAGS = ["actA", "actB", "actA", "actB", "actE", "actA", "actB", "actA", "actB"]

NSLICES = [slice(0, 512), slice(512, 1024)]

_CACHED_NC = None


def _build_module(loops=1):
    nc = bacc.Bacc("TRN2", target_bir_lowering=False, debug=False, num_devices=N_CORES)

    xT = nc.dram_tensor("xT", [D_IN // 128, 128, R], F32, kind="ExternalInput").ap()
    w_aps, b_aps = [], []
    for li, (_, _, fi, fo, _) in enumerate(LAYERS):
        ks, ms = fi // 128, fo // 128
        wdt = F32R if LAYER_F32R[li] else F32
        w_aps.append(
            nc.dram_tensor(f"w{li}", [ms, 128, ks, 128], wdt, kind="ExternalInput").ap()
        )
        b_aps.append(nc.dram_tensor(f"b{li}", [128, ms], F32, kind="ExternalInput").ap())
    crhs = nc.dram_tensor("crhs", [D_EMB // 128, 128, K], F32, kind="ExternalInput").ap()
    c2b = nc.dram_tensor("c2b", [128, K], F32, kind="ExternalInput").ap()

    recT_o = nc.dram_tensor("recT", [D_IN // 128, 128, R], F32, kind="ExternalOutput").ap()
    embT_o = nc.dram_tensor("embT", [D_EMB // 128, 128, R], F32, kind="ExternalOutput").ap()
    lab_o = nc.dram_tensor("labels", [R // 128, 128, K], I32, kind="ExternalOutput").ap()

    with tile.TileContext(nc) as tc, ExitStack() as ctx:
        acts = ctx.enter_context(tc.tile_pool(name="acts", bufs=1))
        wpool = ctx.enter_context(tc.tile_pool(name="wpool", bufs=3))
        bpool = ctx.enter_context(tc.tile_pool(name="bpool", bufs=2))
        psum = ctx.enter_context(tc.tile_pool(name="psum", bufs=6, space="PSUM"))
        cb = ctx.enter_context(tc.tile_pool(name="cb", bufs=1))
        dpool = ctx.enter_context(tc.tile_pool(name="dpool", bufs=2))
        lpool = ctx.enter_context(tc.tile_pool(name="lpool", bufs=2))
        mpool = ctx.enter_context(tc.tile_pool(name="mpool", bufs=2))

        def mlp_layer(li, a_in, rep):
            _, _, fi, fo, relu = LAYERS[li]
            ks, ms = fi // 128, fo // 128
            adt = F32R if ACT_OUT_F32R[li] else F32
            wdt = F32R if LAYER_F32R[li] else F32
            a_out = acts.tile(
                [128, ms, R], adt, tag=ACT_TAGS[li + 1], name=f"act{li}_{rep}"
            )
            bt = bpool.tile([128, ms], F32, tag="bias", name=f"b{li}_{rep}_sb")
            nc.sync.dma_start(bt[:], b_aps[li][:])
            func = (
                mybir.ActivationFunctionType.Relu
                if relu
                else mybir.ActivationFunctionType.Identity
            )
            for m in range(ms):
                wt = wpool.tile([128, ks, 128], wdt, tag="w", name=f"w{li}_{m}_{rep}")
                nc.sync.dma_start(wt[:], w_aps[li][m])
                for n in range(2):
                    ps = psum.tile(
                        [128, 512], F32, tag="ps", name=f"ps{li}_{m}_{n}_{rep}"
                    )
                    for k in range(ks):
                        nc.tensor.matmul(
                            ps[:],
                            wt[:, k, :],
                            a_in[:, k, NSLICES[n]],
                            start=(k == 0),
                            stop=(k == ks - 1),
                        )
                    nc.scalar.activation(
                        a_out[:, m, NSLICES[n]], ps[:], func, bias=bt[:, m : m + 1]
                    )
            return a_out

        for rep in range(loops):
            # input activations, feature-major
            a_x = acts.tile([128, D_IN // 128, R], F32, tag=ACT_TAGS[0], name=f"x_sb_{rep}")
            for c in range(D_IN // 128):
                nc.sync.dma_start(a_x[:, c, :], xT[c])

            # codebook constants
            crhs_sb = cb.tile([128, D_EMB // 128, K], F32, tag="crhs", name=f"crhs_sb_{rep}")
            for k in range(D_EMB // 128):
                nc.sync.dma_start(crhs_sb[:, k, :], crhs[k])
            c2b_sb = cb.tile([128, K], F32, tag="c2b", name=f"c2b_sb_{rep}")
            nc.sync.dma_start(c2b_sb[:], c2b[:])

            h = a_x
            for li in range(4):
                h = mlp_layer(li, h, rep)
            emb = h

            # embedding output
            for k in range(D_EMB // 128):
                nc.sync.dma_start(embT_o[k], emb[:, k, :])

            # codebook: score[row, center] = c2[center] - 2*e.c (argmin-equivalent)
            for m in range(R // 128):
                dist = dpool.tile([128, K], F32, tag="dist", name=f"dist{m}_{rep}")
                for n in range(2):
                    ps = psum.tile([128, 512], F32, tag="ps", name=f"cps{m}_{n}_{rep}")
                    for k in range(D_EMB // 128):
                        nc.tensor.matmul(
                            ps[:],
                            emb[:, k, m * 128 : (m + 1) * 128],
                            crhs_sb[:, k, NSLICES[n]],
                            start=(k == 0),
                            stop=(k == D_EMB // 128 - 1),
                        )
                    nc.vector.tensor_add(
                        dist[:, NSLICES[n]], ps[:], c2b_sb[:, NSLICES[n]]
                    )
                mn = mpool.tile([128, 1], F32, tag="mn", name=f"mn{m}_{rep}")
                nc.vector.tensor_reduce(
                    mn[:], dist[:], axis=mybir.AxisListType.X, op=mybir.AluOpType.min
                )
                lab = lpool.tile([128, K], I32, tag="lab", name=f"lab{m}_{rep}")
                nc.vector.tensor_scalar(
                    lab[:], dist[:], mn[:], None, mybir.AluOpType.is_equal
                )
                nc.sync.dma_start(lab_o[m], lab[:])

            # decoder (f32r): emb stays fp32 for the codebook + output, the
            # decoder reads an f32r-rounded copy
            emb_r = acts.tile(
                [128, D_EMB // 128, R], F32R, tag="embr", name=f"emb_r_{rep}"
            )
            nc.vector.tensor_copy(emb_r[:], emb[:])
            h = emb_r
            for li in range(4, 8):
                h = mlp_layer(li, h, rep)
            rec = h
            for c in range(D_IN // 128):
                nc.sync.dma_start(recT_o[c], rec[:, c, :])

    nc.compile()
    return nc


def _get_nc():
    global _CACHED_NC
    if _CACHED_NC is None:
        _CACHED_NC = _build_module()
    return _CACHED_NC


def _host_prep(inputs):
    """Build the DMA-friendly host-side arrays shared by all cores."""
    shared = {}
    for li, (wk, bk, fi, fo, _) in enumerate(LAYERS):
        ks, ms = fi // 128, fo // 128
        W = np.ascontiguousarray(np.asarray(inputs[wk], dtype=np.float32))
        if LAYER_F32R[li]:
            # pre-round to f32r's 11-bit mantissa so the DMA'd bits are
            # already fixed points of the hardware rounding
            W = _rne11(W)
        # [ks,128,ms,128] -> [ms,128(part=fan_in%128),ks,128(fan_out chunk)]
        shared[f"w{li}"] = np.ascontiguousarray(
            W.reshape(ks, 128, ms, 128).transpose(2, 1, 0, 3)
        )
        b = np.asarray(inputs[bk], dtype=np.float32)
        shared[f"b{li}"] = np.ascontiguousarray(b.reshape(ms, 128).T)
    C = np.asarray(inputs["centers"], dtype=np.float32)
    c2 = (C * C).sum(axis=1).astype(np.float32)  # [K]
    shared["crhs"] = np.ascontiguousarray(
        (-2.0 * C.T).reshape(D_EMB // 128, 128, K).astype(np.float32)
    )
    shared["c2b"] = np.ascontiguousarray(np.broadcast_to(c2[None, :], (128, K)))
    return shared


def kernel(**inputs):
    x = np.asarray(inputs["x"], dtype=np.float32)
    assert x.shape == (B, D_IN)

    shared = _host_prep(inputs)
    in_maps = []
    for c in range(N_CORES):
        shard = x[c * R : (c + 1) * R]  # [R, D_IN]
        xT = np.ascontiguousarray(shard.T).reshape(D_IN // 128, 128, R)
        m = dict(shared)
        m["xT"] = xT
        in_maps.append(m)

    nc = _get_nc()
    res = run_bass_kernel_spmd(nc, in_maps, core_ids=list(range(N_CORES)))

    recs, embs, labs = [], [], []
    for c in range(N_CORES):
        out = res.results[c]
        recs.append(out["recT"].reshape(D_IN, R).T)  # [R, 1024]
        embs.append(out["embT"].reshape(D_EMB, R).T)  # [R, 256]
        labs.append(out["labels"].reshape(R, K))  # [R, 1024] int32
    reconstruction = np.ascontiguousarray(np.concatenate(recs, axis=0), dtype=np.float32)
    embedding = np.ascontiguousarray(np.concatenate(embs, axis=0), dtype=np.float32)
    labels = np.ascontiguousarray(np.concatenate(labs, axis=0), dtype=np.int32)
    return reconstruction, embedding, labels


# revision 15
# speedup vs baseline: 2.0971x; 1.2362x over previous
"""Trainium2 Bass kernel for the VQ-autoencoder problem (nn_DCN_9474697855729).

Reference computation (see problem statement):
  embedding      = MLP_enc(x)          # 1024 -> 2048 -> 1024 -> 512 -> 256, relu on hidden
  reconstruction = MLP_dec(embedding)  # 256 -> 512 -> 1024 -> 2048 -> 1024, relu on hidden
  labels         = one_hot(argmin_k ||embedding - centers_k||^2)

Sharding: data-parallel over batch across 8 NeuronCores (weights + centers
replicated). Each core processes R = 8192/8 = 1024 rows.

Device layout: activations are kept feature-major ("transposed", [feat, row])
in SBUF so that every layer is out[f_out, r] = W[f_in, f_out].T @ h[f_in, r]
with the DRAM weights already in lhsT orientation. The host pre-transposes
x and the weights into DMA-friendly layouts (free on CPU) and undoes the
transpose on the outputs.

For the codebook: argmin_k (e2 + c2 - 2 e.c) == argmin_k (c2 - 2 e.c), so we
matmul emb.T against (-2*centers.T) into [row, center] tiles, add the
precomputed c2 row (replicated across partitions on the host), reduce-min
along the free axis and emit labels = (score == rowmin) as int32.
"""

import numpy as np
from contextlib import ExitStack

import concourse.bass as bass
import concourse.tile as tile
from concourse import bacc, mybir
from concourse.bass_utils import run_bass_kernel_spmd

F32 = mybir.dt.float32
F32R = mybir.dt.float32r
I32 = mybir.dt.int32

B, D_IN, D_EMB, K = 8192, 1024, 256, 1024
N_CORES = 8
R = B // N_CORES  # rows per core

# (input_key_W, input_key_b, fan_in, fan_out, relu)
LAYERS = [
    ("We0", "be0", 1024, 2048, True),
    ("We1", "be1", 2048, 1024, True),
    ("We2", "be2", 1024, 512, True),
    ("We3", "be3", 512, 256, False),
    ("Wd0", "bd0", 256, 512, True),
    ("Wd1", "bd1", 512, 1024, True),
    ("Wd2", "bd2", 1024, 2048, True),
    ("Wd3", "bd3", 2048, 1024, False),
]

# Matmul precision per layer. The encoder (layers 0-3) must stay exact fp32:
# the codebook argmin needs the embedding accurate to ~1e-6 (min top-2
# relative gap measured at 1.3e-5). The decoder only feeds the
# reconstruction output, where float32r's ~1.3e-4 relative error is well
# within tolerance — and f32r matmuls run at 4x the fp32 rate.
# f32r semantics (measured): both operands RNE-rounded to 11 mantissa bits,
# exact multiply, fp32 PSUM accumulate. The BIR verifier requires f32r
# matmul operands to be produced as f32r, so decoder hidden activations are
# written as f32r by the ACT engine and decoder weights are host-rounded.
LAYER_F32R = [False, False, False, False, True, True, True, True]
# Whether layer li's OUTPUT activation tile is written as f32r (True when the
# next layer's matmul is f32r and the tile isn't needed at full precision).
ACT_OUT_F32R = [False, False, False, False, True, True, True, False]


def _rne11(x):
    """Round fp32 array to 11 explicit mantissa bits (f32r's input rounding)."""
    xi = np.ascontiguousarray(x, dtype=np.float32).view(np.uint32)
    shift = np.uint32(12)
    mask = np.uint32(0xFFFFFFFF) << shift
    add = np.uint32(1 << 11)
    lsb = (xi >> shift) & np.uint32(1)
    out = ((xi + add - np.uint32(1) + lsb) & mask).view(np.float32)
    return out

# activation tile tags cycle so non-overlapping lifetimes share SBUF slots
ACT_TAGS = ["actA", "actB", "actA", "actB", "actE", "actA", "actB", "actA", "actB"]

NSLICES = [slice(0, 512), slice(512, 1024)]

_CACHED_NC = None


def _build_module(loops=1):
    nc = bacc.Bacc("TRN2", target_bir_lowering=False, debug=False, num_devices=N_CORES)

    xT = nc.dram_tensor("xT", [D_IN // 128, 128, R], F32, kind="ExternalInput").ap()
    w_aps, b_aps = [], []
    for li, (_, _, fi, fo, _) in enumerate(LAYERS):
        ks, ms = fi // 128, fo // 128
        wdt = F32R if LAYER_F32R[li] else F32
        w_aps.append(
            nc.dram_tensor(f"w{li}", [ms, 128, ks, 128], wdt, kind="ExternalInput").ap()
        )
        b_aps.append(nc.dram_tensor(f"b{li}", [128, ms], F32, kind="ExternalInput").ap())
    crhs = nc.dram_tensor("crhs", [D_EMB // 128, 128, K], F32, kind="ExternalInput").ap()
    c2b = nc.dram_tensor("c2b", [128, K], F32, kind="ExternalInput").ap()

    recT_o = nc.dram_tensor("recT", [D_IN // 128, 128, R], F32, kind="ExternalOutput").ap()
    embT_o = nc.dram_tensor("embT", [D_EMB // 128, 128, R], F32, kind="ExternalOutput").ap()
    lab_o = nc.dram_tensor("labels", [R // 128, 128, K], I32, kind="ExternalOutput").ap()

    with tile.TileContext(nc) as tc, ExitStack() as ctx:
        acts = ctx.enter_context(tc.tile_pool(name="acts", bufs=1))
        wpool = ctx.enter_context(tc.tile_pool(name="wpool", bufs=3))
        bpool = ctx.enter_context(tc.tile_pool(name="bpool", bufs=2))
        psum = ctx.enter_context(tc.tile_pool(name="psum", bufs=6, space="PSUM"))
        cb = ctx.enter_context(tc.tile_pool(name="cb", bufs=1))
        dpool = ctx.enter_context(tc.tile_pool(name="dpool", bufs=2))
        lpool = ctx.enter_context(tc.tile_pool(name="lpool", bufs=2))
        mpool = ctx.enter_context(tc.tile_pool(name="mpool", bufs=2))

        def mlp_layer(li, a_in, rep):
            _, _, fi, fo, relu = LAYERS[li]
            ks, ms = fi // 128, fo // 128
            adt = F32R if ACT_OUT_F32R[li] else F32
            wdt = F32R if LAYER_F32R[li] else F32
            a_out = acts.tile(
                [128, ms, R], adt, tag=ACT_TAGS[li + 1], name=f"act{li}_{rep}"
            )
            bt = bpool.tile([128, ms], F32, tag="bias", name=f"b{li}_{rep}_sb")
            nc.sync.dma_start(bt[:], b_aps[li][:])
            func = (
                mybir.ActivationFunctionType.Relu
                if relu
                else mybir.ActivationFunctionType.Identity
            )
            for m in range(ms):
                wt = wpool.tile([128, ks, 128], wdt, tag="w", name=f"w{li}_{m}_{rep}")
                nc.sync.dma_start(wt[:], w_aps[li][m])
                for n in range(2):
                    ps = psum.tile(
                        [128, 512], F32, tag="ps", name=f"ps{li}_{m}_{n}_{rep}"
                    )
                    for k in range(ks):
                        nc.tensor.matmul(
                            ps[:],
                            wt[:, k, :],
                            a_in[:, k, NSLICES[n]],
                            start=(k == 0),
                            stop=(k == ks - 1),
                        )
                    nc.scalar.activation(
                        a_out[:, m, NSLICES[n]], ps[:], func, bias=bt[:, m : m + 1]
                    )
            return a_out

        for rep in range(loops):
            # input activations, feature-major
            a_x = acts.tile([128, D_IN // 128, R], F32, tag=ACT_TAGS[0], name=f"x_sb_{rep}")
            for c in range(D_IN // 128):
                nc.sync.dma_start(a_x[:, c, :], xT[c])

            # codebook constants
            crhs_sb = cb.tile([128, D_EMB // 128, K], F32, tag="crhs", name=f"crhs_sb_{rep}")
            for k in range(D_EMB // 128):
                nc.sync.dma_start(crhs_sb[:, k, :], crhs[k])
            c2b_sb = cb.tile([128, K], F32, tag="c2b", name=f"c2b_sb_{rep}")
            nc.sync.dma_start(c2b_sb[:], c2b[:])

            h = a_x
            for li in range(4):
                h = mlp_layer(li, h, rep)
            emb = h

            # embedding output
            for k in range(D_EMB // 128):
                nc.sync.dma_start(embT_o[k], emb[:, k, :])

            # codebook: score[row, center] = c2[center] - 2*e.c (argmin-equivalent)
            for m in range(R // 128):
                dist = dpool.tile([128, K], F32, tag="dist", name=f"dist{m}_{rep}")
                for n in range(2):
                    ps = psum.tile([128, 512], F32, tag="ps", name=f"cps{m}_{n}_{rep}")
                    for k in range(D_EMB // 128):
                        nc.tensor.matmul(
                            ps[:],
                            emb[:, k, m * 128 : (m + 1) * 128],
                            crhs_sb[:, k, NSLICES[n]],
                            start=(k == 0),
                            stop=(k == D_EMB // 128 - 1),
                        )
                    nc.vector.tensor_add(
                        dist[:, NSLICES[n]], ps[:], c2b_sb[:, NSLICES[n]]
                    )
                mn = mpool.tile([128, 1], F32, tag="mn", name=f"mn{m}_{rep}")
                nc.vector.tensor_reduce(
                    mn[:], dist[:], axis=mybir.AxisListType.X, op=mybir.AluOpType.min
                )
                lab = lpool.tile([128, K], I32, tag="lab", name=f"lab{m}_{rep}")
                nc.vector.tensor_scalar(
                    lab[:], dist[:], mn[:], None, mybir.AluOpType.is_equal
                )
                nc.sync.dma_start(lab_o[m], lab[:])

            # decoder (f32r): emb stays fp32 for the codebook + output, the
            # decoder reads an f32r-rounded copy
            emb_r = acts.tile(
                [128, D_EMB // 128, R], F32R, tag="embr", name=f"emb_r_{rep}"
            )
            nc.vector.tensor_copy(emb_r[:], emb[:])
            h = emb_r
            for li in range(4, 8):
                h = mlp_layer(li, h, rep)
            rec = h
            for c in range(D_IN // 128):
                nc.sync.dma_start(recT_o[c], rec[:, c, :])

    nc.compile()
    return nc


def _get_nc():
    global _CACHED_NC
    if _CACHED_NC is None:
        _CACHED_NC = _build_module()
    return _CACHED_NC


def _host_prep(inputs):
    """Build the DMA-friendly host-side arrays shared by all cores."""
    shared = {}
    for li, (wk, bk, fi, fo, _) in enumerate(LAYERS):
        ks, ms = fi // 128, fo // 128
        W = np.ascontiguousarray(np.asarray(inputs[wk], dtype=np.float32))
        if LAYER_F32R[li]:
            # pre-round to f32r's 11-bit mantissa so the DMA'd bits are
            # already fixed points of the hardware rounding
            W = _rne11(W)
        # [ks,128,ms,128] -> [ms,128(part=fan_in%128),ks,128(fan_out chunk)]
        shared[f"w{li}"] = np.ascontiguousarray(
            W.reshape(ks, 128, ms, 128).transpose(2, 1, 0, 3)
        )
        b = np.asarray(inputs[bk], dtype=np.float32)
        shared[f"b{li}"] = np.ascontiguousarray(b.reshape(ms, 128).T)
    C = np.asarray(inputs["centers"], dtype=np.float32)
    c2 = (C * C).sum(axis=1).astype(np.float32)  # [K]
    shared["crhs"] = np.ascontiguousarray(
        (-2.0 * C.T).reshape(D_EMB // 128, 128, K).astype(np.float32)
    )
    shared["c2b"] = np.ascontiguousarray(np.broadcast_to(c2[None, :], (128, K)))
    return shared


def _make_in_maps(inputs):
    x = np.asarray(inputs["x"], dtype=np.float32)
    assert x.shape == (B, D_IN)
    shared = _host_prep(inputs)
    in_maps = []
    for c in range(N_CORES):
        shard = x[c * R : (c + 1) * R]  # [R, D_IN]
        xT = np.ascontiguousarray(shard.T).reshape(D_IN // 128, 128, R)
        m = dict(shared)
        m["xT"] = xT
        in_maps.append(m)
    return in_maps


def kernel(**inputs):
    in_maps = _make_in_maps(inputs)
    nc = _get_nc()
    res = run_bass_kernel_spmd(nc, in_maps, core_ids=list(range(N_CORES)))

    recs, embs, labs = [], [], []
    for c in range(N_CORES):
        out = res.results[c]
        recs.append(out["recT"].reshape(D_IN, R).T)  # [R, 1024]
        embs.append(out["embT"].reshape(D_EMB, R).T)  # [R, 256]
        labs.append(out["labels"].reshape(R, K))  # [R, 1024] int32
    reconstruction = np.ascontiguousarray(np.concatenate(recs, axis=0), dtype=np.float32)
    embedding = np.ascontiguousarray(np.concatenate(embs, axis=0), dtype=np.float32)
    labels = np.ascontiguousarray(np.concatenate(labs, axis=0), dtype=np.int32)
    return reconstruction, embedding, labels


# revision 18
# speedup vs baseline: 2.4625x; 1.1742x over previous
"""V2: split-f32r encoder + codebook, pure-f32r decoder, n-chunked (512 rows).

f32r matmuls RNE-round both operands to 11 mantissa bits, multiply exactly and
accumulate in fp32, at 4x the fp32 matmul rate (measured). The encoder needs
~1e-6 accuracy for the codebook argmin, so each fp32 value x is split as
x = xh + xl (both 11-bit f32r numbers, together ~22-23 bits of mantissa) and
each matmul becomes 3 f32r matmuls (Wh@ah + Wh@al + Wl@ah; the dropped Wl@al
term is ~2^-22 relative): 3 cycles/row instead of fp32's 4, at fp32-like
precision. The decoder only feeds the reconstruction and runs single-pass
f32r (~1.3e-4 relative error).

Rows are processed in two chunks of 512 per core so the doubled (hi+lo)
activation tiles fit in SBUF; weights stream twice (~114MB/core, hidden
behind ~520us of PE work).
"""

import numpy as np
from contextlib import ExitStack

import concourse.bass as bass
import concourse.tile as tile
from concourse import bacc, mybir
from concourse.bass_utils import run_bass_kernel_spmd  # noqa: F401 (fallback path)

F32 = mybir.dt.float32
F32R = mybir.dt.float32r
I32 = mybir.dt.int32

B, D_IN, D_EMB, K = 8192, 1024, 256, 1024
N_CORES = 8
R = B // N_CORES  # rows per core
CH = 512  # row-chunk width
NCH = R // CH

LAYERS = [
    ("We0", "be0", 1024, 2048, True),
    ("We1", "be1", 2048, 1024, True),
    ("We2", "be2", 1024, 512, True),
    ("We3", "be3", 512, 256, False),
    ("Wd0", "bd0", 256, 512, True),
    ("Wd1", "bd1", 512, 1024, True),
    ("Wd2", "bd2", 1024, 2048, True),
    ("Wd3", "bd3", 2048, 1024, False),
]

RELU = mybir.ActivationFunctionType.Relu
IDENT = mybir.ActivationFunctionType.Identity

_CACHED_NC = None


def _rne11(x):
    xi = np.ascontiguousarray(x, dtype=np.float32).view(np.uint32)
    shift = np.uint32(12)
    mask = np.uint32(0xFFFFFFFF) << shift
    add = np.uint32(1 << 11)
    lsb = (xi >> shift) & np.uint32(1)
    return ((xi + add - np.uint32(1) + lsb) & mask).view(np.float32)


def _split11(x):
    xh = _rne11(x)
    xl = _rne11(np.asarray(x, np.float32) - xh)
    return xh, xl


def _build_module(loops=1):
    nc = bacc.Bacc("TRN2", target_bir_lowering=False, debug=False, num_devices=N_CORES)

    xTh = nc.dram_tensor("xTh", [D_IN // 128, 128, R], F32R, kind="ExternalInput").ap()
    xTl = nc.dram_tensor("xTl", [D_IN // 128, 128, R], F32R, kind="ExternalInput").ap()
    wh_aps, wl_aps, b_aps = [], [], []
    for li, (_, _, fi, fo, _) in enumerate(LAYERS):
        ks, ms = fi // 128, fo // 128
        wh_aps.append(
            nc.dram_tensor(f"w{li}h", [ms, 128, ks, 128], F32R, kind="ExternalInput").ap()
        )
        if li < 4:
            wl_aps.append(
                nc.dram_tensor(
                    f"w{li}l", [ms, 128, ks, 128], F32R, kind="ExternalInput"
                ).ap()
            )
        else:
            wl_aps.append(None)
        b_aps.append(nc.dram_tensor(f"b{li}", [128, ms], F32, kind="ExternalInput").ap())
    crhsh = nc.dram_tensor("crhsh", [D_EMB // 128, 128, K], F32R, kind="ExternalInput").ap()
    crhsl = nc.dram_tensor("crhsl", [D_EMB // 128, 128, K], F32R, kind="ExternalInput").ap()
    c2b = nc.dram_tensor("c2b", [128, K], F32, kind="ExternalInput").ap()

    recT_o = nc.dram_tensor("recT", [D_IN // 128, 128, R], F32, kind="ExternalOutput").ap()
    embT_o = nc.dram_tensor("embT", [D_EMB // 128, 128, R], F32, kind="ExternalOutput").ap()
    lab_o = nc.dram_tensor("labels", [R // 128, 128, K], I32, kind="ExternalOutput").ap()

    with tile.TileContext(nc) as tc, ExitStack() as ctx:
        # pair slots for split activations; decoder reuses them (see tags)
        acts = ctx.enter_context(tc.tile_pool(name="acts", bufs=1))
        stage = ctx.enter_context(tc.tile_pool(name="stage", bufs=3))
        wpool = ctx.enter_context(tc.tile_pool(name="wpool", bufs=4))
        bpool = ctx.enter_context(tc.tile_pool(name="bpool", bufs=2))
        psum = ctx.enter_context(tc.tile_pool(name="psum", bufs=6, space="PSUM"))
        cb = ctx.enter_context(tc.tile_pool(name="cb", bufs=1))
        dpool = ctx.enter_context(tc.tile_pool(name="dpool", bufs=2))
        lpool = ctx.enter_context(tc.tile_pool(name="lpool", bufs=2))
        mpool = ctx.enter_context(tc.tile_pool(name="mpool", bufs=2))

        # tag plan (KB/partition): A1/A2=16 (xh/xl, h1h/h1l, dec d0/rec),
        # B1/B2=32 (h0h/h0l, h2h/h2l, dec d1/d2), E*=4 (emb trio)
        ENC_TAGS = [("B1", "B2"), ("A1", "A2"), ("B1", "B2"), ("E1", "E2")]
        DEC_TAGS = ["A1", "B1", "B2", "A2"]  # d0, d1, d2, rec

        def split_enc_layer(li, ah, al, ch, rep):
            _, _, fi, fo, relu = LAYERS[li]
            ks, ms = fi // 128, fo // 128
            t1, t2 = ENC_TAGS[li]
            if li == 3:
                emb = acts.tile([128, ms, CH], F32, tag="E0", name=f"emb_{ch}_{rep}")
            oh = acts.tile([128, ms, CH], F32R, tag=t1, name=f"a{li}h_{ch}_{rep}")
            ol = acts.tile([128, ms, CH], F32R, tag=t2, name=f"a{li}l_{ch}_{rep}")
            bt = bpool.tile([128, ms], F32, tag="bias", name=f"b{li}_{ch}_{rep}")
            nc.sync.dma_start(bt[:], b_aps[li][:])
            func = RELU if relu else IDENT
            for m in range(ms):
                wh = wpool.tile([128, ks, 128], F32R, tag="w", name=f"w{li}h_{m}_{ch}_{rep}")
                nc.sync.dma_start(wh[:], wh_aps[li][m])
                wl = wpool.tile([128, ks, 128], F32R, tag="w", name=f"w{li}l_{m}_{ch}_{rep}")
                nc.sync.dma_start(wl[:], wl_aps[li][m])
                ps = psum.tile([128, CH], F32, tag="ps", name=f"ps{li}_{m}_{ch}_{rep}")
                for k in range(ks):
                    nc.tensor.matmul(ps[:], wh[:, k, :], ah[:, k, :], start=(k == 0), stop=False)
                    nc.tensor.matmul(ps[:], wh[:, k, :], al[:, k, :], start=False, stop=False)
                    nc.tensor.matmul(
                        ps[:], wl[:, k, :], ah[:, k, :], start=False, stop=(k == ks - 1)
                    )
                if li == 3:
                    nc.scalar.activation(emb[:, m, :], ps[:], func, bias=bt[:, m : m + 1])
                    nc.vector.tensor_copy(oh[:, m, :], emb[:, m, :])
                    nc.vector.tensor_tensor(
                        ol[:, m, :], emb[:, m, :], oh[:, m, :].bitcast(F32),
                        mybir.AluOpType.subtract,
                    )
                else:
                    st = stage.tile([128, CH], F32, tag="st", name=f"st{li}_{m}_{ch}_{rep}")
                    nc.scalar.activation(st[:], ps[:], func, bias=bt[:, m : m + 1])
                    nc.vector.tensor_copy(oh[:, m, :], st[:])
                    nc.vector.tensor_tensor(
                        ol[:, m, :], st[:], oh[:, m, :].bitcast(F32),
                        mybir.AluOpType.subtract,
                    )
            if li == 3:
                return oh, ol, emb
            return oh, ol

        def dec_layer(li, a_in, ch, rep):
            _, _, fi, fo, relu = LAYERS[li]
            ks, ms = fi // 128, fo // 128
            adt = F32 if li == 7 else F32R
            a_out = acts.tile(
                [128, ms, CH], adt, tag=DEC_TAGS[li - 4], name=f"act{li}_{ch}_{rep}"
            )
            bt = bpool.tile([128, ms], F32, tag="bias", name=f"b{li}_{ch}_{rep}")
            nc.sync.dma_start(bt[:], b_aps[li][:])
            func = RELU if relu else IDENT
            for m in range(ms):
                wt = wpool.tile([128, ks, 128], F32R, tag="w", name=f"w{li}_{m}_{ch}_{rep}")
                nc.sync.dma_start(wt[:], wh_aps[li][m])
                ps = psum.tile([128, CH], F32, tag="ps", name=f"ps{li}_{m}_{ch}_{rep}")
                for k in range(ks):
                    nc.tensor.matmul(
                        ps[:], wt[:, k, :], a_in[:, k, :], start=(k == 0), stop=(k == ks - 1)
                    )
                nc.scalar.activation(a_out[:, m, :], ps[:], func, bias=bt[:, m : m + 1])
            return a_out

        for rep in range(loops):
            crhsh_sb = cb.tile([128, D_EMB // 128, K], F32R, tag="crhsh", name=f"crhsh_{rep}")
            crhsl_sb = cb.tile([128, D_EMB // 128, K], F32R, tag="crhsl", name=f"crhsl_{rep}")
            for k in range(D_EMB // 128):
                nc.sync.dma_start(crhsh_sb[:, k, :], crhsh[k])
                nc.sync.dma_start(crhsl_sb[:, k, :], crhsl[k])
            c2b_sb = cb.tile([128, K], F32, tag="c2b", name=f"c2b_{rep}")
            nc.sync.dma_start(c2b_sb[:], c2b[:])

            for ch in range(NCH):
                RS = slice(ch * CH, (ch + 1) * CH)
                ah = acts.tile([128, D_IN // 128, CH], F32R, tag="A1", name=f"xh_{ch}_{rep}")
                al = acts.tile([128, D_IN // 128, CH], F32R, tag="A2", name=f"xl_{ch}_{rep}")
                for c in range(D_IN // 128):
                    nc.sync.dma_start(ah[:, c, :], xTh[c][:, RS])
                    nc.sync.dma_start(al[:, c, :], xTl[c][:, RS])

                pair = (ah, al)
                for li in range(3):
                    pair = split_enc_layer(li, pair[0], pair[1], ch, rep)
                embh, embl, emb = split_enc_layer(3, pair[0], pair[1], ch, rep)

                for k in range(D_EMB // 128):
                    nc.sync.dma_start(embT_o[k][:, RS], emb[:, k, :])

                # codebook for this chunk's rows
                for mr in range(CH // 128):
                    dist = dpool.tile([128, K], F32, tag="dist", name=f"dist{ch}_{mr}_{rep}")
                    for n in range(2):
                        NS = slice(n * 512, (n + 1) * 512)
                        ps = psum.tile([128, 512], F32, tag="ps", name=f"cps{ch}_{mr}_{n}_{rep}")
                        rsl = slice(mr * 128, (mr + 1) * 128)
                        for k in range(D_EMB // 128):
                            nc.tensor.matmul(
                                ps[:], embh[:, k, rsl], crhsh_sb[:, k, NS],
                                start=(k == 0), stop=False,
                            )
                            nc.tensor.matmul(
                                ps[:], embh[:, k, rsl], crhsl_sb[:, k, NS],
                                start=False, stop=False,
                            )
                            nc.tensor.matmul(
                                ps[:], embl[:, k, rsl], crhsh_sb[:, k, NS],
                                start=False, stop=(k == D_EMB // 128 - 1),
                            )
                        nc.vector.tensor_add(dist[:, NS], ps[:], c2b_sb[:, NS])
                    mn = mpool.tile([128, 1], F32, tag="mn", name=f"mn{ch}_{mr}_{rep}")
                    nc.vector.tensor_reduce(
                        mn[:], dist[:], axis=mybir.AxisListType.X, op=mybir.AluOpType.min
                    )
                    lab = lpool.tile([128, K], I32, tag="lab", name=f"lab{ch}_{mr}_{rep}")
                    nc.vector.tensor_scalar(
                        lab[:], dist[:], mn[:], None, mybir.AluOpType.is_equal
                    )
                    nc.sync.dma_start(lab_o[ch * (CH // 128) + mr], lab[:])

                # decoder: input is embh = rne11(emb), exactly f32r's rounding
                h = embh
                for li in range(4, 8):
                    h = dec_layer(li, h, ch, rep)
                for c in range(D_IN // 128):
                    nc.sync.dma_start(recT_o[c][:, RS], h[:, c, :])

    nc.compile()
    return nc


def _get_nc():
    global _CACHED_NC
    if _CACHED_NC is None:
        _CACHED_NC = _build_module()
    return _CACHED_NC


def _host_prep(inputs):
    shared = {}
    for li, (wk, bk, fi, fo, _) in enumerate(LAYERS):
        ks, ms = fi // 128, fo // 128
        W = np.ascontiguousarray(np.asarray(inputs[wk], dtype=np.float32))
        if li < 4:
            Wh, Wl = _split11(W)
            shared[f"w{li}h"] = np.ascontiguousarray(
                Wh.reshape(ks, 128, ms, 128).transpose(2, 1, 0, 3)
            )
            shared[f"w{li}l"] = np.ascontiguousarray(
                Wl.reshape(ks, 128, ms, 128).transpose(2, 1, 0, 3)
            )
        else:
            Wh = _rne11(W)
            shared[f"w{li}h"] = np.ascontiguousarray(
                Wh.reshape(ks, 128, ms, 128).transpose(2, 1, 0, 3)
            )
        b = np.asarray(inputs[bk], dtype=np.float32)
        shared[f"b{li}"] = np.ascontiguousarray(b.reshape(ms, 128).T)
    C = np.asarray(inputs["centers"], dtype=np.float32)
    c2 = (C * C).sum(axis=1).astype(np.float32)
    crhs = (-2.0 * C.T).astype(np.float32)  # [D_EMB, K]
    ch_, cl_ = _split11(crhs)
    shared["crhsh"] = np.ascontiguousarray(ch_.reshape(D_EMB // 128, 128, K))
    shared["crhsl"] = np.ascontiguousarray(cl_.reshape(D_EMB // 128, 128, K))
    shared["c2b"] = np.ascontiguousarray(np.broadcast_to(c2[None, :], (128, K)))
    return shared


def _make_in_maps(inputs):
    x = np.asarray(inputs["x"], dtype=np.float32)
    assert x.shape == (B, D_IN)
    shared = _host_prep(inputs)
    in_maps = []
    for c in range(N_CORES):
        shard = x[c * R : (c + 1) * R]
        xT = np.ascontiguousarray(shard.T)
        xh, xl = _split11(xT)
        m = dict(shared)
        m["xTh"] = xh.reshape(D_IN // 128, 128, R)
        m["xTl"] = xl.reshape(D_IN // 128, 128, R)
        in_maps.append(m)
    return in_maps


_CACHED_RUNNER = None


def _get_runner():
    """Jit the 8-core shard_map'd bass_exec once and reuse across calls."""
    global _CACHED_RUNNER
    if _CACHED_RUNNER is not None:
        return _CACHED_RUNNER

    import jax
    from jax.sharding import Mesh, NamedSharding, PartitionSpec
    from jax.experimental.shard_map import shard_map
    from concourse.bass2jax import (
        _bass_exec_p,
        install_neuronx_cc_hook,
        partition_id_tensor,
    )

    nc = _get_nc()
    install_neuronx_cc_hook()
    partition_name = nc.partition_id_tensor.name if nc.partition_id_tensor else None

    in_names, out_names, out_avals, out_shapes = [], [], [], []
    for alloc in nc.m.functions[0].allocations:
        if not isinstance(alloc, mybir.MemoryLocationSet):
            continue
        name = alloc.memorylocations[0].name
        if alloc.kind == "ExternalInput":
            if name != partition_name:
                in_names.append(name)
        elif alloc.kind == "ExternalOutput":
            out_names.append(name)
            shape = tuple(alloc.tensor_shape)
            dtype = mybir.dt.np(alloc.dtype)
            out_avals.append(jax.core.ShapedArray(shape, dtype))
            out_shapes.append((shape, dtype))
    n_params = len(in_names)
    all_in_names = list(in_names) + list(out_names)
    if partition_name is not None:
        all_in_names.append(partition_name)

    def _body(*args):
        operands = list(args)
        if partition_name is not None:
            operands.append(partition_id_tensor())
        return tuple(
            _bass_exec_p.bind(
                *operands,
                out_avals=tuple(out_avals),
                in_names=tuple(all_in_names),
                out_names=tuple(out_names),
                lowering_input_output_aliases=(),
                sim_require_finite=True,
                sim_require_nnan=True,
                nc=nc,
            )
        )

    devices = jax.devices()[:N_CORES]
    mesh = Mesh(np.asarray(devices), ("core",))
    in_specs = (PartitionSpec("core"),) * (n_params + len(out_names))
    out_specs = (PartitionSpec("core"),) * len(out_names)
    fn = jax.jit(
        shard_map(
            _body, mesh=mesh, in_specs=in_specs, out_specs=out_specs, check_rep=False
        ),
        keep_unused=True,
    )
    sharding = NamedSharding(mesh, PartitionSpec("core"))
    _CACHED_RUNNER = (fn, in_names, out_names, out_shapes, sharding)
    return _CACHED_RUNNER


def kernel(**inputs):
    import jax

    in_maps = _make_in_maps(inputs)
    fn, in_names, out_names, out_shapes, sharding = _get_runner()

    concat_in = [
        np.concatenate([np.asarray(in_maps[c][nm]) for c in range(N_CORES)], axis=0)
        for nm in in_names
    ]
    concat_zeros = [
        np.zeros((N_CORES * s[0], *s[1:]), dt) for (s, dt) in out_shapes
    ]
    dev_in = [jax.device_put(a, sharding) for a in concat_in]
    dev_zero = [jax.device_put(a, sharding) for a in concat_zeros]
    out_arrs = fn(*dev_in, *dev_zero)
    outs = {
        name: np.asarray(out_arrs[i]).reshape(N_CORES, *out_shapes[i][0])
        for i, name in enumerate(out_names)
    }

    recs, embs, labs = [], [], []
    for c in range(N_CORES):
        recs.append(outs["recT"][c].reshape(D_IN, R).T)
        embs.append(outs["embT"][c].reshape(D_EMB, R).T)
        labs.append(outs["labels"][c].reshape(R, K))
    reconstruction = np.ascontiguousarray(np.concatenate(recs, axis=0), dtype=np.float32)
    embedding = np.ascontiguousarray(np.concatenate(embs, axis=0), dtype=np.float32)
    labels = np.ascontiguousarray(np.concatenate(labs, axis=0), dtype=np.int32)
    return reconstruction, embedding, labels


# revision 19
# speedup vs baseline: 3.0201x; 1.2264x over previous
"""V2: split-f32r encoder + codebook, pure-f32r decoder, n-chunked (512 rows).

f32r matmuls RNE-round both operands to 11 mantissa bits, multiply exactly and
accumulate in fp32, at 4x the fp32 matmul rate (measured). The encoder needs
~1e-6 accuracy for the codebook argmin, so each fp32 value x is split as
x = xh + xl (both 11-bit f32r numbers, together ~22-23 bits of mantissa) and
each matmul becomes 3 f32r matmuls (Wh@ah + Wh@al + Wl@ah; the dropped Wl@al
term is ~2^-22 relative): 3 cycles/row instead of fp32's 4, at fp32-like
precision. The decoder only feeds the reconstruction and runs single-pass
f32r (~1.3e-4 relative error).

Rows are processed in two chunks of 512 per core so the doubled (hi+lo)
activation tiles fit in SBUF; weights stream twice (~114MB/core, hidden
behind ~520us of PE work).
"""

import numpy as np
from contextlib import ExitStack

import concourse.bass as bass
import concourse.tile as tile
from concourse import bacc, mybir
from concourse.bass_utils import run_bass_kernel_spmd  # noqa: F401 (fallback path)

F32 = mybir.dt.float32
F32R = mybir.dt.float32r
I32 = mybir.dt.int32

B, D_IN, D_EMB, K = 8192, 1024, 256, 1024
N_CORES = 8
R = B // N_CORES  # rows per core
CH = 512  # row-chunk width
NCH = R // CH

LAYERS = [
    ("We0", "be0", 1024, 2048, True),
    ("We1", "be1", 2048, 1024, True),
    ("We2", "be2", 1024, 512, True),
    ("We3", "be3", 512, 256, False),
    ("Wd0", "bd0", 256, 512, True),
    ("Wd1", "bd1", 512, 1024, True),
    ("Wd2", "bd2", 1024, 2048, True),
    ("Wd3", "bd3", 2048, 1024, False),
]

RELU = mybir.ActivationFunctionType.Relu
IDENT = mybir.ActivationFunctionType.Identity

_CACHED_NC = None


def _rne11(x):
    xi = np.ascontiguousarray(x, dtype=np.float32).view(np.uint32)
    shift = np.uint32(12)
    mask = np.uint32(0xFFFFFFFF) << shift
    add = np.uint32(1 << 11)
    lsb = (xi >> shift) & np.uint32(1)
    return ((xi + add - np.uint32(1) + lsb) & mask).view(np.float32)


def _split11(x):
    xh = _rne11(x)
    xl = _rne11(np.asarray(x, np.float32) - xh)
    return xh, xl


def _build_module(loops=1):
    nc = bacc.Bacc("TRN2", target_bir_lowering=False, debug=False, num_devices=N_CORES)

    xTh = nc.dram_tensor("xTh", [D_IN // 128, 128, R], F32R, kind="ExternalInput").ap()
    xTl = nc.dram_tensor("xTl", [D_IN // 128, 128, R], F32R, kind="ExternalInput").ap()
    wh_aps, wl_aps, b_aps = [], [], []
    for li, (_, _, fi, fo, _) in enumerate(LAYERS):
        ks, ms = fi // 128, fo // 128
        wh_aps.append(
            nc.dram_tensor(f"w{li}h", [ms, 128, ks, 128], F32R, kind="ExternalInput").ap()
        )
        if li < 4:
            wl_aps.append(
                nc.dram_tensor(
                    f"w{li}l", [ms, 128, ks, 128], F32R, kind="ExternalInput"
                ).ap()
            )
        else:
            wl_aps.append(None)
        b_aps.append(nc.dram_tensor(f"b{li}", [128, ms], F32, kind="ExternalInput").ap())
    crhsh = nc.dram_tensor("crhsh", [D_EMB // 128, 128, K], F32R, kind="ExternalInput").ap()
    crhsl = nc.dram_tensor("crhsl", [D_EMB // 128, 128, K], F32R, kind="ExternalInput").ap()
    c2b = nc.dram_tensor("c2b", [128, K], F32, kind="ExternalInput").ap()

    recT_o = nc.dram_tensor("recT", [D_IN // 128, 128, R], F32, kind="ExternalOutput").ap()
    embT_o = nc.dram_tensor("embT", [D_EMB // 128, 128, R], F32, kind="ExternalOutput").ap()
    lab_o = nc.dram_tensor("labels", [R // 128, 128, K], I32, kind="ExternalOutput").ap()

    with tile.TileContext(nc) as tc, ExitStack() as ctx:
        # pair slots for split activations; decoder reuses them (see tags)
        acts = ctx.enter_context(tc.tile_pool(name="acts", bufs=1))
        stage = ctx.enter_context(tc.tile_pool(name="stage", bufs=3))
        wpool = ctx.enter_context(tc.tile_pool(name="wpool", bufs=4))
        bpool = ctx.enter_context(tc.tile_pool(name="bpool", bufs=2))
        psum = ctx.enter_context(tc.tile_pool(name="psum", bufs=8, space="PSUM"))
        cb = ctx.enter_context(tc.tile_pool(name="cb", bufs=1))
        dpool = ctx.enter_context(tc.tile_pool(name="dpool", bufs=2))
        lpool = ctx.enter_context(tc.tile_pool(name="lpool", bufs=2))
        mpool = ctx.enter_context(tc.tile_pool(name="mpool", bufs=2))

        # tag plan (KB/partition): A1/A2=16 (xh/xl, h1h/h1l, dec d0/rec),
        # B1/B2=32 (h0h/h0l, h2h/h2l, dec d1/d2), E*=4 (emb trio)
        ENC_TAGS = [("B1", "B2"), ("A1", "A2"), ("B1", "B2"), ("E1", "E2")]
        DEC_TAGS = ["A1", "B1", "B2", "A2"]  # d0, d1, d2, rec

        def split_enc_layer(li, ah, al, ch, rep):
            _, _, fi, fo, relu = LAYERS[li]
            ks, ms = fi // 128, fo // 128
            t1, t2 = ENC_TAGS[li]
            if li == 3:
                emb = acts.tile([128, ms, CH], F32, tag="E0", name=f"emb_{ch}_{rep}")
            oh = acts.tile([128, ms, CH], F32R, tag=t1, name=f"a{li}h_{ch}_{rep}")
            ol = acts.tile([128, ms, CH], F32R, tag=t2, name=f"a{li}l_{ch}_{rep}")
            bt = bpool.tile([128, ms], F32, tag="bias", name=f"b{li}_{ch}_{rep}")
            nc.sync.dma_start(bt[:], b_aps[li][:])
            func = RELU if relu else IDENT
            for m in range(ms):
                wh = wpool.tile([128, ks, 128], F32R, tag="w", name=f"w{li}h_{m}_{ch}_{rep}")
                nc.sync.dma_start(wh[:], wh_aps[li][m])
                wl = wpool.tile([128, ks, 128], F32R, tag="w", name=f"w{li}l_{m}_{ch}_{rep}")
                nc.sync.dma_start(wl[:], wl_aps[li][m])
                ps = psum.tile([128, CH], F32, tag="ps", name=f"ps{li}_{m}_{ch}_{rep}")
                for k in range(ks):
                    nc.tensor.matmul(ps[:], wh[:, k, :], ah[:, k, :], start=(k == 0), stop=False)
                    nc.tensor.matmul(ps[:], wh[:, k, :], al[:, k, :], start=False, stop=False)
                    nc.tensor.matmul(
                        ps[:], wl[:, k, :], ah[:, k, :], start=False, stop=(k == ks - 1)
                    )
                if li == 3:
                    nc.scalar.activation(emb[:, m, :], ps[:], func, bias=bt[:, m : m + 1])
                    nc.vector.tensor_copy(oh[:, m, :], emb[:, m, :])
                    nc.vector.tensor_tensor(
                        ol[:, m, :], emb[:, m, :], oh[:, m, :].bitcast(F32),
                        mybir.AluOpType.subtract,
                    )
                else:
                    st = stage.tile([128, CH], F32, tag="st", name=f"st{li}_{m}_{ch}_{rep}")
                    nc.scalar.activation(st[:], ps[:], func, bias=bt[:, m : m + 1])
                    nc.vector.tensor_copy(oh[:, m, :], st[:])
                    nc.vector.tensor_tensor(
                        ol[:, m, :], st[:], oh[:, m, :].bitcast(F32),
                        mybir.AluOpType.subtract,
                    )
            if li == 3:
                return oh, ol, emb
            return oh, ol

        def dec_layer(li, a_in, ch, rep):
            _, _, fi, fo, relu = LAYERS[li]
            ks, ms = fi // 128, fo // 128
            adt = F32 if li == 7 else F32R
            a_out = acts.tile(
                [128, ms, CH], adt, tag=DEC_TAGS[li - 4], name=f"act{li}_{ch}_{rep}"
            )
            bt = bpool.tile([128, ms], F32, tag="bias", name=f"b{li}_{ch}_{rep}")
            nc.sync.dma_start(bt[:], b_aps[li][:])
            func = RELU if relu else IDENT
            for m in range(ms):
                wt = wpool.tile([128, ks, 128], F32R, tag="w", name=f"w{li}_{m}_{ch}_{rep}")
                nc.sync.dma_start(wt[:], wh_aps[li][m])
                ps = psum.tile([128, CH], F32, tag="ps", name=f"ps{li}_{m}_{ch}_{rep}")
                for k in range(ks):
                    nc.tensor.matmul(
                        ps[:], wt[:, k, :], a_in[:, k, :], start=(k == 0), stop=(k == ks - 1)
                    )
                nc.scalar.activation(a_out[:, m, :], ps[:], func, bias=bt[:, m : m + 1])
            return a_out

        for rep in range(loops):
            crhsh_sb = cb.tile([128, D_EMB // 128, K], F32R, tag="crhsh", name=f"crhsh_{rep}")
            crhsl_sb = cb.tile([128, D_EMB // 128, K], F32R, tag="crhsl", name=f"crhsl_{rep}")
            for k in range(D_EMB // 128):
                nc.sync.dma_start(crhsh_sb[:, k, :], crhsh[k])
                nc.sync.dma_start(crhsl_sb[:, k, :], crhsl[k])
            c2b_sb = cb.tile([128, K], F32, tag="c2b", name=f"c2b_{rep}")
            nc.sync.dma_start(c2b_sb[:], c2b[:])

            for ch in range(NCH):
                RS = slice(ch * CH, (ch + 1) * CH)
                ah = acts.tile([128, D_IN // 128, CH], F32R, tag="A1", name=f"xh_{ch}_{rep}")
                al = acts.tile([128, D_IN // 128, CH], F32R, tag="A2", name=f"xl_{ch}_{rep}")
                for c in range(D_IN // 128):
                    nc.sync.dma_start(ah[:, c, :], xTh[c][:, RS])
                    nc.sync.dma_start(al[:, c, :], xTl[c][:, RS])

                pair = (ah, al)
                for li in range(3):
                    pair = split_enc_layer(li, pair[0], pair[1], ch, rep)
                embh, embl, emb = split_enc_layer(3, pair[0], pair[1], ch, rep)

                for k in range(D_EMB // 128):
                    nc.sync.dma_start(embT_o[k][:, RS], emb[:, k, :])

                # codebook for this chunk's rows
                for mr in range(CH // 128):
                    dist = dpool.tile([128, K], F32, tag="dist", name=f"dist{ch}_{mr}_{rep}")
                    for n in range(2):
                        NS = slice(n * 512, (n + 1) * 512)
                        ps = psum.tile([128, 512], F32, tag="ps", name=f"cps{ch}_{mr}_{n}_{rep}")
                        rsl = slice(mr * 128, (mr + 1) * 128)
                        for k in range(D_EMB // 128):
                            nc.tensor.matmul(
                                ps[:], embh[:, k, rsl], crhsh_sb[:, k, NS],
                                start=(k == 0), stop=False,
                            )
                            nc.tensor.matmul(
                                ps[:], embh[:, k, rsl], crhsl_sb[:, k, NS],
                                start=False, stop=False,
                            )
                            nc.tensor.matmul(
                                ps[:], embl[:, k, rsl], crhsh_sb[:, k, NS],
                                start=False, stop=(k == D_EMB // 128 - 1),
                            )
                        nc.vector.tensor_add(dist[:, NS], ps[:], c2b_sb[:, NS])
                    mn = mpool.tile([128, 1], F32, tag="mn", name=f"mn{ch}_{mr}_{rep}")
                    nc.vector.tensor_reduce(
                        mn[:], dist[:], axis=mybir.AxisListType.X, op=mybir.AluOpType.min
                    )
                    lab = lpool.tile([128, K], I32, tag="lab", name=f"lab{ch}_{mr}_{rep}")
                    nc.vector.tensor_scalar(
                        lab[:], dist[:], mn[:], None, mybir.AluOpType.is_equal
                    )
                    nc.sync.dma_start(lab_o[ch * (CH // 128) + mr], lab[:])

                # decoder: input is embh = rne11(emb), exactly f32r's rounding
                h = embh
                for li in range(4, 8):
                    h = dec_layer(li, h, ch, rep)
                for c in range(D_IN // 128):
                    nc.sync.dma_start(recT_o[c][:, RS], h[:, c, :])

    nc.compile()
    return nc


def _get_nc():
    global _CACHED_NC
    if _CACHED_NC is None:
        _CACHED_NC = _build_module()
    return _CACHED_NC


def _host_prep(inputs):
    shared = {}
    for li, (wk, bk, fi, fo, _) in enumerate(LAYERS):
        ks, ms = fi // 128, fo // 128
        W = np.ascontiguousarray(np.asarray(inputs[wk], dtype=np.float32))
        if li < 4:
            Wh, Wl = _split11(W)
            shared[f"w{li}h"] = np.ascontiguousarray(
                Wh.reshape(ks, 128, ms, 128).transpose(2, 1, 0, 3)
            )
            shared[f"w{li}l"] = np.ascontiguousarray(
                Wl.reshape(ks, 128, ms, 128).transpose(2, 1, 0, 3)
            )
        else:
            Wh = _rne11(W)
            shared[f"w{li}h"] = np.ascontiguousarray(
                Wh.reshape(ks, 128, ms, 128).transpose(2, 1, 0, 3)
            )
        b = np.asarray(inputs[bk], dtype=np.float32)
        shared[f"b{li}"] = np.ascontiguousarray(b.reshape(ms, 128).T)
    C = np.asarray(inputs["centers"], dtype=np.float32)
    c2 = (C * C).sum(axis=1).astype(np.float32)
    crhs = (-2.0 * C.T).astype(np.float32)  # [D_EMB, K]
    ch_, cl_ = _split11(crhs)
    shared["crhsh"] = np.ascontiguousarray(ch_.reshape(D_EMB // 128, 128, K))
    shared["crhsl"] = np.ascontiguousarray(cl_.reshape(D_EMB // 128, 128, K))
    shared["c2b"] = np.ascontiguousarray(np.broadcast_to(c2[None, :], (128, K)))
    return shared


def _make_in_maps(inputs):
    x = np.asarray(inputs["x"], dtype=np.float32)
    assert x.shape == (B, D_IN)
    shared = _host_prep(inputs)
    in_maps = []
    for c in range(N_CORES):
        shard = x[c * R : (c + 1) * R]
        xT = np.ascontiguousarray(shard.T)
        xh, xl = _split11(xT)
        m = dict(shared)
        m["xTh"] = xh.reshape(D_IN // 128, 128, R)
        m["xTl"] = xl.reshape(D_IN // 128, 128, R)
        in_maps.append(m)
    return in_maps


_CACHED_RUNNER = None


def _get_runner():
    """Jit the 8-core shard_map'd bass_exec once and reuse across calls."""
    global _CACHED_RUNNER
    if _CACHED_RUNNER is not None:
        return _CACHED_RUNNER

    import jax
    from jax.sharding import Mesh, NamedSharding, PartitionSpec
    from jax.experimental.shard_map import shard_map
    from concourse.bass2jax import (
        _bass_exec_p,
        install_neuronx_cc_hook,
        partition_id_tensor,
    )

    nc = _get_nc()
    install_neuronx_cc_hook()
    partition_name = nc.partition_id_tensor.name if nc.partition_id_tensor else None

    in_names, out_names, out_avals, out_shapes = [], [], [], []
    for alloc in nc.m.functions[0].allocations:
        if not isinstance(alloc, mybir.MemoryLocationSet):
            continue
        name = alloc.memorylocations[0].name
        if alloc.kind == "ExternalInput":
            if name != partition_name:
                in_names.append(name)
        elif alloc.kind == "ExternalOutput":
            out_names.append(name)
            shape = tuple(alloc.tensor_shape)
            dtype = mybir.dt.np(alloc.dtype)
            out_avals.append(jax.core.ShapedArray(shape, dtype))
            out_shapes.append((shape, dtype))
    n_params = len(in_names)
    all_in_names = list(in_names) + list(out_names)
    if partition_name is not None:
        all_in_names.append(partition_name)

    def _body(*args):
        operands = list(args)
        if partition_name is not None:
            operands.append(partition_id_tensor())
        return tuple(
            _bass_exec_p.bind(
                *operands,
                out_avals=tuple(out_avals),
                in_names=tuple(all_in_names),
                out_names=tuple(out_names),
                lowering_input_output_aliases=(),
                sim_require_finite=True,
                sim_require_nnan=True,
                nc=nc,
            )
        )

    devices = jax.devices()[:N_CORES]
    mesh = Mesh(np.asarray(devices), ("core",))
    in_specs = (PartitionSpec("core"),) * (n_params + len(out_names))
    out_specs = (PartitionSpec("core"),) * len(out_names)
    fn = jax.jit(
        shard_map(
            _body, mesh=mesh, in_specs=in_specs, out_specs=out_specs, check_rep=False
        ),
        keep_unused=True,
    )
    sharding = NamedSharding(mesh, PartitionSpec("core"))
    _CACHED_RUNNER = (fn, in_names, out_names, out_shapes, sharding)
    return _CACHED_RUNNER


def kernel(**inputs):
    import jax

    in_maps = _make_in_maps(inputs)
    fn, in_names, out_names, out_shapes, sharding = _get_runner()

    concat_in = [
        np.concatenate([np.asarray(in_maps[c][nm]) for c in range(N_CORES)], axis=0)
        for nm in in_names
    ]
    concat_zeros = [
        np.zeros((N_CORES * s[0], *s[1:]), dt) for (s, dt) in out_shapes
    ]
    dev_in = [jax.device_put(a, sharding) for a in concat_in]
    dev_zero = [jax.device_put(a, sharding) for a in concat_zeros]
    out_arrs = fn(*dev_in, *dev_zero)
    outs = {
        name: np.asarray(out_arrs[i]).reshape(N_CORES, *out_shapes[i][0])
        for i, name in enumerate(out_names)
    }

    recs, embs, labs = [], [], []
    for c in range(N_CORES):
        recs.append(outs["recT"][c].reshape(D_IN, R).T)
        embs.append(outs["embT"][c].reshape(D_EMB, R).T)
        labs.append(outs["labels"][c].reshape(R, K))
    reconstruction = np.ascontiguousarray(np.concatenate(recs, axis=0), dtype=np.float32)
    embedding = np.ascontiguousarray(np.concatenate(embs, axis=0), dtype=np.float32)
    labels = np.ascontiguousarray(np.concatenate(labs, axis=0), dtype=np.int32)
    return reconstruction, embedding, labels
